# revision 6
# baseline (speedup 1.0000x reference)
"""AdaIN statistics kernel for TRN2, SPMD across 8 NeuronCores. v3.

Input : f_vol [32, 512, 64, 64] f32
Output: [32, 1024] f32 = concat([mean over (h,w), unbiased std over (h,w)], axis=-1)

Sharding: data-parallel over batch - each of the 8 cores handles 4 batches
([4, 512, 64, 64] shard, 32 MiB). No collectives; the host concatenates the
8 per-core [4, 1024] outputs.

v3 key change vs v2: engine-load rebalancing. Trace analysis of v2 showed
SDMA engine 15 (serving SBUF partitions 92-95/124-127) systematically runs
at ~21 B/ns while engines 0-14 run at ~26.7 B/ns (fabric line rate; the
sibling NC is idle so the per-core stream is fabric-limited at ~436 GB/s,
not HBM-limited at 358).  With a uniform 16-rows-per-partition deal, engine
15 finishes ~20 us after everyone else and sets the critical path.

New deal: "slow" lanes (ports 7 and 15: partitions 76-79/108-111 and
92-95/124-127) carry 13 rows; other lanes carry 16, and 48 designated
"mini" lanes (spread evenly over the 12 remaining ports with minis:
{0..31} u {64..75} u {96..99}) carry 17.  Engine bytes: slow ports
104 rows, fast ports <= 132 rows -> both finish at ~80-82 us.

Slab schedule (issue order = per-engine FIFO arrival order):
  u0..u3  uniform m=2 slabs, rows [0,1024): 1 DMA each, ring slots 0-3.
  a0..a2  asymmetric slabs, rows [1280,2000): per slab 8 DMAs
          (F1 lanes 0-75 m=2; 4 slow 4-lane blocks m=1; 3 fast 12-lane
          blocks m=2), ring slots 0-2 (reusing u0-u2 after consumption).
  minis   rows [2000,2048) -> 48 mini lanes, 1 row, 3 DMAs into xt2.
  u4      uniform slab rows [1024,1280), ring slot 4, split into 9
          column-chunk DMAs (7x512, 384, 128 cols) so DVE pipelines
          bn_stats against chunk arrivals; only the final 128-col chunk's
          stats + epilogue + out-DMA are exposed after the last byte.

Compute split: ACT consumes u1,u2,u3 via Copy/Square+accumulate; DVE does
bn_stats/bn_aggr for u0, a0-a2, minis, u4.  All output DMAs are issued
from the SYNC queue after all input issues (ACT stays under the stream
time; sync has ~45 us of issue work against an ~82 us stream).

Hard-won semaphore lesson (from v2): same-engine RAW through SBUF is NOT
covered by program order.  Every producer->consumer edge carries an
explicit semaphore observation.  SWDGE requires sem values to start at 0,
so semaphores are never reused; stats/mv/res buffers are never reused.
"""

from contextlib import ExitStack

import numpy as np

B, C, H, W = 32, 512, 64, 64
N_CORES = 8
B_LOCAL = B // N_CORES  # 4
N = H * W  # 4096
P = 128
ROWS = B_LOCAL * C  # 2048

# lane classes
SLOW = [76 + 16 * k + j for k in range(4) for j in range(4)]  # ports 7,15
MINI = list(range(0, 32)) + list(range(64, 76)) + list(range(96, 100))
MINI_RUNS = [(0, 32), (64, 12), (96, 4)]  # (base lane, count), rows in order
assert len(MINI) == 48 and not (set(MINI) & set(SLOW))

NU = 5            # uniform slabs (u4 is the chunked one)
NA = 3            # asymmetric slabs
A_BASE = 1280     # first row of asym slabs; u slabs cover [0,1280)
A_ROWS = 240      # rows per asym slab
M_BASE = 2000     # first mini row
U4_BASE = 1024    # rows of the chunked uniform slab
CHUNKS = [512] * 7 + [384, 128]  # u4 column chunks
assert sum(CHUNKS) == N
U_SLOTS = {0: 0, 1: 1, 2: 2, 3: 3, 4: 4}
A_SLOTS = {0: 0, 1: 1, 2: 2}

_CACHE = {}


def _coverage_check(pieces):
    """pieces: list of (row, lane, slot) - assert rows 0..2047 covered once."""
    seen = {}
    for r, lane, slot in pieces:
        assert r not in seen, f"row {r} dealt twice"
        seen[r] = (lane, slot)
    assert len(seen) == ROWS, (len(seen), ROWS)


def _build():
    import concourse.bass as bass
    from concourse import mybir

    nc = bass.Bass()
    x_ext = nc.declare_dram_parameter(
        "f_vol", [B_LOCAL, C, H, W], mybir.dt.float32, isOutput=False
    )
    out_ext = nc.declare_dram_parameter(
        "out", [B_LOCAL, 2 * C], mybir.dt.float32, isOutput=True
    )

    x = x_ext.ap().rearrange("b c h w -> (b c) (h w)")  # [2048, 4096]

    # ---- build-time coverage audit ----
    pieces = []
    for j in range(4):
        for p in range(P):
            for m in range(2):
                pieces.append((256 * j + 2 * p + m, p, 2 * j + m))
    for p in range(P):
        for m in range(2):
            pieces.append((U4_BASE + 2 * p + m, p, 8 + m))
    for t in range(NA):
        o = A_BASE + A_ROWS * t
        for p in range(76):
            for m in range(2):
                pieces.append((o + 2 * p + m, p, 10 + 2 * t + m))
        for k in range(4):
            for j in range(4):
                pieces.append((o + 152 + 28 * k + j, 76 + 16 * k + j, 10 + t))
        for k in range(3):
            for q in range(12):
                for m in range(2):
                    pieces.append(
                        (o + 156 + 28 * k + 2 * q + m, 80 + 16 * k + q, 10 + 2 * t + m)
                    )
    for i, lane in enumerate(MINI):
        pieces.append((M_BASE + i, lane, 16))
    _coverage_check(pieces)

    # ---- semaphore count plans (cumulative values, by emission order) ----
    # dve_stats: +1 per DVE bn_stats.  DVE order: u0 (16), a0 (16), a1, a2,
    # minis (24), u4 chunks (2 each, 18).
    dve_cum = {"u0": 16, "a0": 32, "a1": 48, "a2": 64, "m": 88}
    dve_u4 = {g: 88 + 2 * (g + 1) for g in range(len(CHUNKS))}
    DVE_TOTAL = dve_u4[len(CHUNKS) - 1]
    # mv_ready: +1 per bn_aggr, same order (2 per slab, 3 for minis, 2 for u4)
    mv_after = {"u0": 2, "a0": 4, "a1": 6, "a2": 8, "m": 11, "u4": 13}
    # act_stats: +1 per ACT accumulate pass (Copy or Square); u1, u2, u3 get
    # 4 each (2 rows x 2 passes).
    acts_after = {1: 4, 2: 8, 3: 12}

    with ExitStack() as ctx:
        block = ctx.enter_context(nc.Block(no_gpsimd_drain=True))
        dma_u = [ctx.enter_context(nc.semaphore(f"dma_u{j}")) for j in range(4)]
        dma_a = [ctx.enter_context(nc.semaphore(f"dma_a{t}")) for t in range(NA)]
        dma_m = ctx.enter_context(nc.semaphore("dma_m"))
        dma_c = [
            ctx.enter_context(nc.semaphore(f"dma_c{g}")) for g in range(len(CHUNKS))
        ]
        out_sem = ctx.enter_context(nc.semaphore("out_sem"))
        dve_stats = ctx.enter_context(nc.semaphore("dve_stats"))
        mv_ready = ctx.enter_context(nc.semaphore("mv_ready"))
        act_stats = ctx.enter_context(nc.semaphore("act_stats"))
        act_done = ctx.enter_context(nc.semaphore("act_done"))
        warm_done = ctx.enter_context(nc.semaphore("warm_done"))

        xt = ctx.enter_context(nc.sbuf_tensor("xt", [P, NU, 2 * N], mybir.dt.float32))
        xt2 = ctx.enter_context(nc.sbuf_tensor("xt2", [P, N], mybir.dt.float32))
        # stats: u-slot 0 = u0, u-slot 1 = u4 (9 groups); asym 8 groups/row
        stats_u = ctx.enter_context(
            nc.sbuf_tensor("stats_u", [P, 2, 2, 9, 6], mybir.dt.float32)
        )
        stats_a = ctx.enter_context(
            nc.sbuf_tensor("stats_a", [P, NA, 2, 8, 6], mybir.dt.float32)
        )
        stats_m = ctx.enter_context(
            nc.sbuf_tensor("stats_m", [P, 8, 6], mybir.dt.float32)
        )
        mv_u = ctx.enter_context(nc.sbuf_tensor("mv_u", [P, 2, 2, 2], mybir.dt.float32))
        mv_a = ctx.enter_context(
            nc.sbuf_tensor("mv_a", [P, NA, 2, 2], mybir.dt.float32)
        )
        mv_m = ctx.enter_context(nc.sbuf_tensor("mv_m", [P, 2], mybir.dt.float32))
        res_u = ctx.enter_context(
            nc.sbuf_tensor("res_u", [P, NU, 2, 2], mybir.dt.float32)
        )
        res_a = ctx.enter_context(
            nc.sbuf_tensor("res_a", [P, NA, 2, 2], mybir.dt.float32)
        )
        res_m = ctx.enter_context(nc.sbuf_tensor("res_m", [P, 2], mybir.dt.float32))
        acc = ctx.enter_context(nc.sbuf_tensor("acc", [P, 3, 2, 3], mybir.dt.float32))
        warm = ctx.enter_context(nc.sbuf_tensor("warm", [P, 2], mybir.dt.float32))

        # act_done cumulative gates per epilogue, in ACT emission order:
        # epi(u0)=4 mv-form ops; epi(u1),(u2),(u3)=8 acc-form ops each;
        # epi(a0..a2)=4 each; epi(minis)=6; epi(u4)=4.
        actd = {}
        cact = 0
        for name, n in [
            ("u0", 4), ("u1", 8), ("u2", 8), ("u3", 8),
            ("a0", 4), ("a1", 4), ("a2", 4), ("m", 6), ("u4", 4),
        ]:
            cact += n
            actd[name] = cact
        ACT_TOTAL = cact

        # ---- output DMA plan: (gate_name, src_ap_fn, dst_ap) list ----
        # built inside sync block below; count for the final wait:
        N_OUTS = 5 + (8 * NA + 1) + 3  # uniform + asym (t=1 F1 split) + minis

        @block.sync
        def _(sync):
            # u0..u3: uniform m=2 slabs
            for j in range(4):
                src = x[256 * j : 256 * j + 256, :].rearrange(
                    "(p m) f -> p (m f)", m=2
                )
                sync.dma_start(out=xt[:, U_SLOTS[j], :], in_=src).then_inc(
                    dma_u[j], 16
                )
            # a0..a2: asymmetric slabs; ring slots 0-2 reuse u0-u2
            for t in range(NA):
                o = A_BASE + A_ROWS * t
                s = A_SLOTS[t]
                if t == 0:
                    sync.wait_ge(dve_stats, dve_cum["u0"])  # u0 consumed
                else:
                    sync.wait_ge(act_stats, acts_after[t])  # u_t consumed
                src = x[o : o + 152, :].rearrange("(p m) f -> p (m f)", m=2)
                sync.dma_start(out=xt[0:76, s, :], in_=src).then_inc(dma_a[t], 16)
                for k in range(4):
                    r0 = o + 152 + 28 * k
                    sync.dma_start(
                        out=xt[76 + 16 * k : 80 + 16 * k, s, 0:N],
                        in_=x[r0 : r0 + 4, :],
                    ).then_inc(dma_a[t], 16)
                for k in range(3):
                    r0 = o + 156 + 28 * k
                    src = x[r0 : r0 + 24, :].rearrange("(p m) f -> p (m f)", m=2)
                    sync.dma_start(
                        out=xt[80 + 16 * k : 92 + 16 * k, s, :], in_=src
                    ).then_inc(dma_a[t], 16)
            # minis
            i0 = 0
            for base, cnt in MINI_RUNS:
                src = x[M_BASE + i0 : M_BASE + i0 + cnt, :]
                sync.dma_start(out=xt2[base : base + cnt, :], in_=src).then_inc(
                    dma_m, 16
                )
                i0 += cnt
            # u4: column-chunked uniform slab, ring slot 4 (fresh)
            xs4 = x[U4_BASE : U4_BASE + 256, :].rearrange("(p m) f -> p m f", m=2)
            xd4 = xt[:, U_SLOTS[4], :].rearrange("p (m f) -> p m f", f=N)
            c0 = 0
            for g, w in enumerate(CHUNKS):
                sync.dma_start(
                    out=xd4[:, :, c0 : c0 + w], in_=xs4[:, :, c0 : c0 + w]
                ).then_inc(dma_c[g], 16)
                c0 += w

            # ---- output DMAs (after all input issues; FIFO keeps them
            # behind the input descriptors per engine, which is fine) ----
            def out_u(j):
                b = (256 * j) // 512
                dst = bass.AP(
                    tensor=out_ext,
                    offset=b * 2 * C + (256 * j - 512 * b),
                    ap=[[2, P], [C, 2], [1, 2]],
                )
                sync.dma_start(out=dst, in_=res_u[:, j, :, :]).then_inc(out_sem, 16)

            def out_a(t):
                o = A_BASE + A_ROWS * t
                # F1 lanes 0-75 (split at batch boundary for t=1)
                f1_splits = [(0, 76)] if t != 1 else [(0, 8), (8, 68)]
                for p0, np_ in f1_splits:
                    r0 = o + 2 * p0
                    b = r0 // 512
                    dst = bass.AP(
                        tensor=out_ext,
                        offset=b * 2 * C + (r0 - 512 * b),
                        ap=[[2, np_], [C, 2], [1, 2]],
                    )
                    sync.dma_start(
                        out=dst, in_=res_a[p0 : p0 + np_, t, :, :]
                    ).then_inc(out_sem, 16)
                for k in range(4):
                    r0 = o + 152 + 28 * k
                    b = r0 // 512
                    assert (r0 + 3) // 512 == b
                    dst = bass.AP(
                        tensor=out_ext,
                        offset=b * 2 * C + (r0 - 512 * b),
                        ap=[[1, 4], [C, 2], [1, 1]],
                    )
                    with nc.allow_non_contiguous_dma(reason="16B scattered stats"):
                        sync.dma_start(
                            out=dst, in_=res_a[76 + 16 * k : 80 + 16 * k, t, :, 0:1]
                        ).then_inc(out_sem, 16)
                for k in range(3):
                    r0 = o + 156 + 28 * k
                    b = r0 // 512
                    assert (r0 + 23) // 512 == b
                    dst = bass.AP(
                        tensor=out_ext,
                        offset=b * 2 * C + (r0 - 512 * b),
                        ap=[[2, 12], [C, 2], [1, 2]],
                    )
                    sync.dma_start(
                        out=dst, in_=res_a[80 + 16 * k : 92 + 16 * k, t, :, :]
                    ).then_inc(out_sem, 16)

            def out_m():
                i0 = 0
                for base, cnt in MINI_RUNS:
                    r0 = M_BASE + i0
                    b = r0 // 512
                    dst = bass.AP(
                        tensor=out_ext,
                        offset=b * 2 * C + (r0 - 512 * b),
                        ap=[[1, cnt], [C, 2], [1, 1]],
                    )
                    with nc.allow_non_contiguous_dma(reason="16B scattered stats"):
                        sync.dma_start(
                            out=dst,
                            in_=res_m[base : base + cnt, :].unsqueeze(2),
                        ).then_inc(out_sem, 16)
                    i0 += cnt

            sync.wait_ge(act_done, actd["u0"])
            out_u(0)
            sync.wait_ge(act_done, actd["u1"])
            out_u(1)
            sync.wait_ge(act_done, actd["u2"])
            out_u(2)
            sync.wait_ge(act_done, actd["u3"])
            out_u(3)
            sync.wait_ge(act_done, actd["a0"])
            out_a(0)
            sync.wait_ge(act_done, actd["a1"])
            out_a(1)
            sync.wait_ge(act_done, actd["a2"])
            out_a(2)
            sync.wait_ge(act_done, actd["m"])
            out_m()
            sync.wait_ge(act_done, actd["u4"])
            out_u(4)
            sync.wait_ge(out_sem, 16 * N_OUTS)

        @block.vector
        def _(vector):
            ndve = 0
            nmv = 0

            vector.memset(warm[:, :], 0.0).then_inc(warm_done, 1)

            def slab_u0():
                nonlocal ndve, nmv
                vector.wait_ge(dma_u[0], 16)
                for r in range(2):
                    for g in range(8):
                        vector.bn_stats(
                            out=stats_u[:, 0, r, g, :],
                            in_=xt[:, 0, (r * 8 + g) * 512 : (r * 8 + g + 1) * 512],
                        ).then_inc(dve_stats, 1)
                        ndve += 1
                vector.wait_ge(dve_stats, ndve)
                for r in range(2):
                    vector.bn_aggr(
                        out=mv_u[:, 0, r, :], in_=stats_u[:, 0, r, 0:8, :]
                    ).then_inc(mv_ready, 1)
                    nmv += 1

            def slab_a(t):
                nonlocal ndve, nmv
                vector.wait_ge(dma_a[t], 16 * 8)
                s = A_SLOTS[t]
                for r in range(2):
                    for g in range(8):
                        vector.bn_stats(
                            out=stats_a[:, t, r, g, :],
                            in_=xt[:, s, (r * 8 + g) * 512 : (r * 8 + g + 1) * 512],
                        ).then_inc(dve_stats, 1)
                        ndve += 1
                vector.wait_ge(dve_stats, ndve)
                for r in range(2):
                    vector.bn_aggr(
                        out=mv_a[:, t, r, :], in_=stats_a[:, t, r, :, :]
                    ).then_inc(mv_ready, 1)
                    nmv += 1

            def slab_m():
                nonlocal ndve, nmv
                vector.wait_ge(dma_m, 16 * 3)
                for base, cnt in MINI_RUNS:
                    for g in range(8):
                        vector.bn_stats(
                            out=stats_m[base : base + cnt, g, :],
                            in_=xt2[base : base + cnt, g * 512 : (g + 1) * 512],
                        ).then_inc(dve_stats, 1)
                        ndve += 1
                vector.wait_ge(dve_stats, ndve)
                for base, cnt in MINI_RUNS:
                    vector.bn_aggr(
                        out=mv_m[base : base + cnt, :],
                        in_=stats_m[base : base + cnt, :, :],
                    ).then_inc(mv_ready, 1)
                    nmv += 1

            def slab_u4():
                nonlocal ndve, nmv
                c0 = 0
                for g, w in enumerate(CHUNKS):
                    vector.wait_ge(dma_c[g], 16)
                    for r in range(2):
                        vector.bn_stats(
                            out=stats_u[:, 1, r, g, :],
                            in_=xt[:, U_SLOTS[4], r * N + c0 : r * N + c0 + w],
                        ).then_inc(dve_stats, 1)
                        ndve += 1
                    c0 += w
                vector.wait_ge(dve_stats, ndve)
                for r in range(2):
                    vector.bn_aggr(
                        out=mv_u[:, 1, r, :], in_=stats_u[:, 1, r, :, :]
                    ).then_inc(mv_ready, 1)
                    nmv += 1

            slab_u0()
            assert ndve == dve_cum["u0"] and nmv == mv_after["u0"]
            for t in range(NA):
                slab_a(t)
                assert ndve == dve_cum[f"a{t}"] and nmv == mv_after[f"a{t}"]
            slab_m()
            assert ndve == dve_cum["m"] and nmv == mv_after["m"]
            slab_u4()
            assert ndve == DVE_TOTAL and nmv == mv_after["u4"]

        @block.scalar
        def _(scalar):
            A = 1.0 / np.sqrt(float(N) * (N - 1))
            cact = 0
            nacc = 0

            # warm the ACT function table while the first slab streams
            scalar.wait_ge(warm_done, 1)
            scalar.activation(
                out=warm[:, 0:1],
                in_=warm[:, 1:2],
                func=mybir.ActivationFunctionType.Copy,
            )

            def acc_pass(j):
                # j in {1,2,3}: consume uniform slab j via accumulate
                nonlocal nacc
                scalar.wait_ge(dma_u[j], 16)
                for r in range(2):
                    row = xt[:, U_SLOTS[j], r * N : (r + 1) * N]
                    scalar.activation(
                        out=row,
                        in_=row,
                        func=mybir.ActivationFunctionType.Copy,
                        accum_out=acc[:, j - 1, r, 0:1],
                    ).then_inc(act_stats, 1)
                    nacc += 1
                    scalar.wait_ge(act_stats, nacc)  # Copy retired before Square
                    scalar.activation(
                        out=row,
                        in_=row,
                        func=mybir.ActivationFunctionType.Square,
                        accum_out=acc[:, j - 1, r, 1:2],
                    ).then_inc(act_stats, 1)
                    nacc += 1

            def mv_stat_ops(mean_src, var_src, mean_dst, std_dst):
                nonlocal cact
                scalar.copy(out=mean_dst, in_=mean_src).then_inc(act_done, 1)
                scalar.activation(
                    out=std_dst,
                    in_=var_src,
                    func=mybir.ActivationFunctionType.Sqrt,
                    scale=float(N) / (N - 1),
                ).then_inc(act_done, 1)
                cact += 2

            def acc_stat_ops(aslab, r, mean_dst, std_dst):
                nonlocal cact
                scalar.activation(
                    out=mean_dst,
                    in_=acc[:, aslab, r, 0:1],
                    func=mybir.ActivationFunctionType.Copy,
                    scale=1.0 / N,
                ).then_inc(act_done, 1)
                scalar.activation(
                    out=acc[:, aslab, r, 2:3],
                    in_=acc[:, aslab, r, 0:1],
                    func=mybir.ActivationFunctionType.Square,
                    scale=A,
                ).then_inc(act_done, 1)
                cact += 2
                scalar.wait_ge(act_done, cact)
                scalar.activation(
                    out=acc[:, aslab, r, 2:3],
                    in_=acc[:, aslab, r, 2:3],
                    func=mybir.ActivationFunctionType.Copy,
                    scale=-1.0,
                ).then_inc(act_done, 1)
                cact += 1
                scalar.wait_ge(act_done, cact)
                scalar.activation(
                    out=std_dst,
                    in_=acc[:, aslab, r, 1:2],
                    func=mybir.ActivationFunctionType.Sqrt,
                    scale=1.0 / (N - 1),
                    bias=acc[:, aslab, r, 2:3],
                ).then_inc(act_done, 1)
                cact += 1

            def epi_u(j):
                if j in (0, 4):
                    scalar.wait_ge(mv_ready, mv_after["u0" if j == 0 else "u4"])
                    slot = 0 if j == 0 else 1
                    for r in range(2):
                        mv_stat_ops(
                            mv_u[:, slot, r, 0:1],
                            mv_u[:, slot, r, 1:2],
                            res_u[:, j, 0, r : r + 1],
                            res_u[:, j, 1, r : r + 1],
                        )
                else:
                    scalar.wait_ge(act_stats, acts_after[j])
                    for r in range(2):
                        acc_stat_ops(
                            j - 1,
                            r,
                            res_u[:, j, 0, r : r + 1],
                            res_u[:, j, 1, r : r + 1],
                        )

            def epi_a(t):
                scalar.wait_ge(mv_ready, mv_after[f"a{t}"])
                for r in range(2):
                    mv_stat_ops(
                        mv_a[:, t, r, 0:1],
                        mv_a[:, t, r, 1:2],
                        res_a[:, t, 0, r : r + 1],
                        res_a[:, t, 1, r : r + 1],
                    )

            def epi_m():
                scalar.wait_ge(mv_ready, mv_after["m"])
                for base, cnt in MINI_RUNS:
                    mv_stat_ops(
                        mv_m[base : base + cnt, 0:1],
                        mv_m[base : base + cnt, 1:2],
                        res_m[base : base + cnt, 0:1],
                        res_m[base : base + cnt, 1:2],
                    )

            acc_pass(1)
            epi_u(0)
            assert cact == actd["u0"]
            acc_pass(2)
            epi_u(1)
            assert cact == actd["u1"]
            acc_pass(3)
            epi_u(2)
            assert cact == actd["u2"]
            epi_u(3)
            assert cact == actd["u3"]
            epi_a(0)
            epi_a(1)
            epi_a(2)
            assert cact == actd["a2"]
            epi_m()
            assert cact == actd["m"]
            epi_u(4)
            assert cact == actd["u4"] == ACT_TOTAL

    return nc


def kernel(f_vol: np.ndarray) -> np.ndarray:
    from concourse.bass_utils import run_bass_kernel_spmd

    if "nc" not in _CACHE:
        _CACHE["nc"] = _build()
    nc = _CACHE["nc"]

    f_vol = np.ascontiguousarray(f_vol, dtype=np.float32)
    in_maps = [
        {"f_vol": f_vol[i * B_LOCAL : (i + 1) * B_LOCAL]} for i in range(N_CORES)
    ]
    res = run_bass_kernel_spmd(nc, in_maps, core_ids=list(range(N_CORES)))
    return np.concatenate([res.results[i]["out"] for i in range(N_CORES)], axis=0)


# revision 10
# speedup vs baseline: 2.0111x; 2.0111x over previous
"""AdaIN statistics kernel for TRN2, SPMD across 8 NeuronCores. v4.

Input : f_vol [32, 512, 64, 64] f32
Output: [32, 1024] f32 = concat([mean over (h,w), unbiased std over (h,w)], axis=-1)

Sharding: data-parallel over batch - each of the 8 cores handles 4 batches
([4, 512, 64, 64] shard, 32 MiB). No collectives; the host concatenates the
8 per-core [4, 1024] outputs.

v4 design, from trace evidence of v2 (uniform interleaved deal, per-slab
scattered out-DMAs) and v3 (partition-rebalanced deal):
  - Input DMAs must be uniform 128-partition transfers with large (32 KiB)
    per-partition descriptors: those ran at full per-engine line rate
    (~26.7 B/ns) on ALL 16 queue rings in v3.  Sub-128-partition DMAs get
    concentrated onto engine quads (v3's asym slabs overloaded engines
    0-3 2.5x) and small descriptors run at half rate.
  - v2's outputs were ~1800 scattered 8-byte descriptors (per-slab
    [[2,P],[C,2],[1,2]] APs); v2 showed one ring running ~21% slow with
    exactly the per-slab periodicity of those outs.  v4 eliminates them.

Lane-major deal: lane p owns rows 16p..16p+15 (row = b*512+c of the
[2048, 4096] row-major view).  Slab s (s=0..7) moves rows {16p+2s,
16p+2s+1}: per-partition 32 KiB contiguous, one clean DMA.  Outputs:
means/stds accumulate on-chip into MS[128, 2, 16] (lane p, mean|std,
slot); because lane p's rows are 16 consecutive c-positions of batch
p//32, the whole output is expressible as TWO DMAs:
    dst AP [[1024, 4], [16, 32], [512, 2], [1, w]]  (2-level partition dim)
an early one for slots 0..13 (hidden mid-stream) and a final one for
slots 14..15.

The last slab (s=7) is column-chunked (2048/1024/512/384/128 cols) so DVE
pipelines bn_stats against chunk arrivals; only the final 128-col chunk's
2 bn_stats + 2 bn_aggr + 4 ACT ops + 1 tiny out-DMA trail the last byte.

Compute split: ACT consumes slabs s1,s2,s3 via Copy/Square+accumulate
(freeing DVE to keep up with the stream); DVE does bn_stats/bn_aggr for
s0, s4, s5, s6, s7.  Ring: 5 xt slots; s5->slot0 (after s0 consumed by
DVE), s6->slot1 (after s1: ACT), s7->slot2 (after s2: ACT).

Hard-won semaphore lesson (v2): same-engine RAW through SBUF is NOT
covered by program order.  Every producer->consumer edge carries an
explicit semaphore observation.  SWDGE requires sem values to start at 0,
so semaphores are never reused; stats/mv/res buffers are never reused.
"""

from contextlib import ExitStack

import numpy as np

B, C, H, W = 32, 512, 64, 64
N_CORES = 8
B_LOCAL = B // N_CORES  # 4
N = H * W  # 4096
P = 128
ROWS = B_LOCAL * C  # 2048
RPL = ROWS // P  # 16 rows per lane

NBUF = 5
NSLAB = 8  # slabs of 2 rows/lane
CHUNKS = [2048, 1024, 512, 384, 128]  # column chunks of the last slab
assert sum(CHUNKS) == N
# bn_stats group widths per row for the chunked slab (FMAX=512)
GROUPS7 = [512] * 7 + [384, 128]
DVE_SLABS = (0, 4, 5, 6)  # full slabs on DVE (s7 chunked, also DVE)
ACT_SLABS = (1, 2, 3)  # slabs consumed by ACT accumulate
EARLY_SLOTS = 14  # slots 0..13 in the early out-DMA; 14..15 in the final

_CACHE = {}


def _build():
    import concourse.bass as bass
    from concourse import mybir

    nc = bass.Bass()
    x_ext = nc.declare_dram_parameter(
        "f_vol", [B_LOCAL, C, H, W], mybir.dt.float32, isOutput=False
    )
    out_ext = nc.declare_dram_parameter(
        "out", [B_LOCAL, 2 * C], mybir.dt.float32, isOutput=True
    )

    # [128, 16*4096]: lane p <- rows 16p..16p+15, contiguous per lane
    xl = (
        x_ext.ap()
        .rearrange("b c h w -> (b c) (h w)")
        .rearrange("(p u) f -> p (u f)", u=RPL)
    )

    # semaphore plans (cumulative, by emission order)
    # dve_stats: s0 (16), s4 (16), s5, s6, then chunks of s7 (2 per chunk)
    dve_cum = {0: 16, 4: 32, 5: 48, 6: 64}
    DVE_TOTAL = 64 + 2 * len(CHUNKS)
    # mv_ready: 2 per DVE slab in the same order, +2 for s7
    mv_after = {0: 2, 4: 4, 5: 6, 6: 8, 7: 10}
    # act_stats: +1 per ACT accumulate pass; s1, s2, s3 get 4 each
    acts_after = {1: 4, 2: 8, 3: 12}

    with ExitStack() as ctx:
        block = ctx.enter_context(nc.Block(no_gpsimd_drain=True))
        dma_s = [ctx.enter_context(nc.semaphore(f"dma_s{s}")) for s in range(7)]
        dma_c = [
            ctx.enter_context(nc.semaphore(f"dma_c{g}")) for g in range(len(CHUNKS))
        ]
        out_sem = ctx.enter_context(nc.semaphore("out_sem"))
        dve_stats = ctx.enter_context(nc.semaphore("dve_stats"))
        mv_ready = ctx.enter_context(nc.semaphore("mv_ready"))
        act_stats = ctx.enter_context(nc.semaphore("act_stats"))
        act_done = ctx.enter_context(nc.semaphore("act_done"))
        warm_done = ctx.enter_context(nc.semaphore("warm_done"))

        xt = ctx.enter_context(nc.sbuf_tensor("xt", [P, NBUF, 2 * N], mybir.dt.float32))
        # stats slot per DVE slab: 0,4,5,6 -> 0..3; s7 -> 4 (9 groups)
        stats = ctx.enter_context(
            nc.sbuf_tensor("stats", [P, 5, 2, 9, 6], mybir.dt.float32)
        )
        mv = ctx.enter_context(nc.sbuf_tensor("mv", [P, 5, 2, 2], mybir.dt.float32))
        # output image: [lane, mean|std, slot]
        MS = ctx.enter_context(nc.sbuf_tensor("MS", [P, 2, RPL], mybir.dt.float32))
        acc = ctx.enter_context(nc.sbuf_tensor("acc", [P, 3, 2, 3], mybir.dt.float32))
        warm = ctx.enter_context(nc.sbuf_tensor("warm", [P, 2], mybir.dt.float32))

        sslot = {0: 0, 4: 1, 5: 2, 6: 3, 7: 4}  # stats/mv slot per DVE slab
        ring = {s: s % NBUF for s in range(NSLAB)}  # s5->0, s6->1, s7->2

        # act_done cumulative gates, in ACT emission order:
        # epi(s0)=4 mv-form; epi(s1..s3)=8 acc-form; epi(s4..s6)=4; epi(s7)=4
        actd = {}
        cact_plan = 0
        for s, n in [(0, 4), (1, 8), (2, 8), (3, 8), (4, 4), (5, 4), (6, 4), (7, 4)]:
            cact_plan += n
            actd[s] = cact_plan
        ACT_TOTAL = cact_plan

        def slab_src(s):
            return xl[:, 2 * s * N : (2 * s + 2) * N]

        @block.sync
        def _(sync):
            for s in range(7):
                if s >= NBUF:
                    sp = s - NBUF
                    if sp in ACT_SLABS:
                        sync.wait_ge(act_stats, acts_after[sp])
                    else:
                        sync.wait_ge(dve_stats, dve_cum[sp])
                sync.dma_start(out=xt[:, ring[s], :], in_=slab_src(s)).then_inc(
                    dma_s[s], 16
                )
            # s7: column-chunked into ring slot 2 (s2 is ACT-consumed)
            sync.wait_ge(act_stats, acts_after[2])
            xs7 = slab_src(7).rearrange("p (m f) -> p m f", f=N)
            xd7 = xt[:, ring[7], :].rearrange("p (m f) -> p m f", f=N)
            c0 = 0
            for g, w in enumerate(CHUNKS):
                sync.dma_start(
                    out=xd7[:, :, c0 : c0 + w], in_=xs7[:, :, c0 : c0 + w]
                ).then_inc(dma_c[g], 16)
                c0 += w
            sync.wait_ge(out_sem, 64)

        @block.vector
        def _(vector):
            ndve = 0
            nmv = 0

            vector.memset(warm[:, :], 0.0).then_inc(warm_done, 1)

            for s in DVE_SLABS:
                ss = sslot[s]
                vector.wait_ge(dma_s[s], 16)
                for r in range(2):
                    for g in range(8):
                        vector.bn_stats(
                            out=stats[:, ss, r, g, :],
                            in_=xt[:, ring[s], (r * 8 + g) * 512 : (r * 8 + g + 1) * 512],
                        ).then_inc(dve_stats, 1)
                        ndve += 1
                assert ndve == dve_cum[s]
                vector.wait_ge(dve_stats, ndve)
                for r in range(2):
                    vector.bn_aggr(
                        out=mv[:, ss, r, :], in_=stats[:, ss, r, 0:8, :]
                    ).then_inc(mv_ready, 1)
                    nmv += 1

            # s7 chunks: bn_stats pipelined against chunk arrivals
            ss = sslot[7]
            c0 = 0
            gi = 0
            for g, w in enumerate(CHUNKS):
                vector.wait_ge(dma_c[g], 16)
                # groups covered by this chunk: consecutive, widths from GROUPS7
                cend = c0 + w
                gg = c0
                while gg < cend:
                    gw = GROUPS7[gi]
                    for r in range(2):
                        vector.bn_stats(
                            out=stats[:, ss, r, gi, :],
                            in_=xt[:, ring[7], r * N + gg : r * N + gg + gw],
                        ).then_inc(dve_stats, 1)
                        ndve += 1
                    gg += gw
                    gi += 1
                assert gg == cend
                c0 = cend
            assert gi == len(GROUPS7)
            # GROUPS7 chunks each contain 2 bn_stats per 512-col group...
            vector.wait_ge(dve_stats, ndve)
            for r in range(2):
                vector.bn_aggr(
                    out=mv[:, ss, r, :], in_=stats[:, ss, r, :, :]
                ).then_inc(mv_ready, 1)
                nmv += 1
            assert nmv == mv_after[7]

        @block.scalar
        def _(scalar):
            A = 1.0 / np.sqrt(float(N) * (N - 1))
            cact = 0
            nacc = 0

            scalar.wait_ge(warm_done, 1)
            scalar.activation(
                out=warm[:, 0:1],
                in_=warm[:, 1:2],
                func=mybir.ActivationFunctionType.Copy,
            )

            def acc_pass(s):
                nonlocal nacc
                ai = s - 1  # acc slot for s in {1,2,3}
                scalar.wait_ge(dma_s[s], 16)
                for r in range(2):
                    row = xt[:, ring[s], r * N : (r + 1) * N]
                    scalar.activation(
                        out=row,
                        in_=row,
                        func=mybir.ActivationFunctionType.Copy,
                        accum_out=acc[:, ai, r, 0:1],
                    ).then_inc(act_stats, 1)
                    nacc += 1
                    scalar.wait_ge(act_stats, nacc)
                    scalar.activation(
                        out=row,
                        in_=row,
                        func=mybir.ActivationFunctionType.Square,
                        accum_out=acc[:, ai, r, 1:2],
                    ).then_inc(act_stats, 1)
                    nacc += 1
                assert nacc == acts_after[s]

            def mv_stat_ops(mean_src, var_src, mean_dst, std_dst):
                nonlocal cact
                scalar.copy(out=mean_dst, in_=mean_src).then_inc(act_done, 1)
                scalar.activation(
                    out=std_dst,
                    in_=var_src,
                    func=mybir.ActivationFunctionType.Sqrt,
                    scale=float(N) / (N - 1),
                ).then_inc(act_done, 1)
                cact += 2

            def acc_stat_ops(ai, r, mean_dst, std_dst):
                nonlocal cact
                scalar.activation(
                    out=mean_dst,
                    in_=acc[:, ai, r, 0:1],
                    func=mybir.ActivationFunctionType.Copy,
                    scale=1.0 / N,
                ).then_inc(act_done, 1)
                scalar.activation(
                    out=acc[:, ai, r, 2:3],
                    in_=acc[:, ai, r, 0:1],
                    func=mybir.ActivationFunctionType.Square,
                    scale=A,
                ).then_inc(act_done, 1)
                cact += 2
                scalar.wait_ge(act_done, cact)
                scalar.activation(
                    out=acc[:, ai, r, 2:3],
                    in_=acc[:, ai, r, 2:3],
                    func=mybir.ActivationFunctionType.Copy,
                    scale=-1.0,
                ).then_inc(act_done, 1)
                cact += 1
                scalar.wait_ge(act_done, cact)
                scalar.activation(
                    out=std_dst,
                    in_=acc[:, ai, r, 1:2],
                    func=mybir.ActivationFunctionType.Sqrt,
                    scale=1.0 / (N - 1),
                    bias=acc[:, ai, r, 2:3],
                ).then_inc(act_done, 1)
                cact += 1

            def epi(s):
                # results into MS[:, 0, 2s+r] (mean) and MS[:, 1, 2s+r] (std)
                if s in ACT_SLABS:
                    scalar.wait_ge(act_stats, acts_after[s])
                    for r in range(2):
                        acc_stat_ops(
                            s - 1,
                            r,
                            MS[:, 0, 2 * s + r : 2 * s + r + 1],
                            MS[:, 1, 2 * s + r : 2 * s + r + 1],
                        )
                else:
                    scalar.wait_ge(mv_ready, mv_after[s])
                    ss = sslot[s]
                    for r in range(2):
                        mv_stat_ops(
                            mv[:, ss, r, 0:1],
                            mv[:, ss, r, 1:2],
                            MS[:, 0, 2 * s + r : 2 * s + r + 1],
                            MS[:, 1, 2 * s + r : 2 * s + r + 1],
                        )
                assert cact == actd[s], (s, cact, actd[s])

            def out_dma(u0, w):
                # one DMA for means, one for stds (APs are limited to 3 dims)
                for is_std in range(2):
                    dst = bass.AP(
                        tensor=out_ext,
                        offset=is_std * C + u0,
                        ap=[[2 * C, B_LOCAL], [RPL, P // B_LOCAL], [1, w]],
                    )
                    scalar.dma_start(
                        out=dst, in_=MS[:, is_std, u0 : u0 + w]
                    ).then_inc(out_sem, 16)

            acc_pass(1)
            acc_pass(2)
            epi(0)
            acc_pass(3)
            epi(1)
            epi(2)
            epi(3)
            epi(4)
            epi(5)
            epi(6)
            # early out: slots 0..13 (slabs 0-6) - hidden mid-stream
            scalar.wait_ge(act_done, actd[6])
            out_dma(0, EARLY_SLOTS)
            epi(7)
            scalar.wait_ge(act_done, actd[7])
            out_dma(EARLY_SLOTS, RPL - EARLY_SLOTS)
            assert cact == ACT_TOTAL

    return nc


def kernel(f_vol: np.ndarray) -> np.ndarray:
    from concourse.bass_utils import run_bass_kernel_spmd

    if "nc" not in _CACHE:
        _CACHE["nc"] = _build()
    nc = _CACHE["nc"]

    f_vol = np.ascontiguousarray(f_vol, dtype=np.float32)
    in_maps = [
        {"f_vol": f_vol[i * B_LOCAL : (i + 1) * B_LOCAL]} for i in range(N_CORES)
    ]
    res = run_bass_kernel_spmd(nc, in_maps, core_ids=list(range(N_CORES)))
    return np.concatenate([res.results[i]["out"] for i in range(N_CORES)], axis=0)
